# revision 18
# baseline (speedup 1.0000x reference)
"""Trainium2 Bass kernel for a 2-layer GRU LM (nn_GRU_19086834664042).

Shapes (hardcoded): V=10000, E=256, H=512, L=2, S=256, B=64.
Sharding: data-parallel over batch B across 8 cores (8 sequences per core).

Layout strategy: the recurrence runs fully "transposed" — gate matmuls are
weight-stationary (lhsT = weight tile, rhs = hidden state [H, B]) so the
gates land in PSUM with the gate dimension on 128 partitions and the tiny
batch (8) in the free dimension. All elementwise work then runs at full
128-lane width (~100ns/op instead of ~1us), and no PE transposes are needed
anywhere. Weights are bf16 (fast weight load); state/pre-activations fp32.

Phases per core:
  P1: gx0T[t] = (X0[t] @ [Wr0x|Wz0x|Wh0x] + b0)^T, one big weight-stationary
      matmul over all 2048 tokens -> DRAM scratch (transposed, step-major).
  P2: recurrence, software-pipelined across layers in 16-step blocks
      (layer-1 block k runs interleaved with layer-0 block k+1):
        rzT = sigmoid(WhrzT-mm(hT) + gx_rzT); rhT = rT*hT;
        cT = tanh(WhcT-mm(rhT) + gx_cT); hT' = hT + zT*(cT - hT)
      gx1T for a block = (h0_block @ W1x + b1)^T, batched matmul.
      h1T history (bf16) accumulates in SBUF as the logits lhsT.
  P3: logits = h1_all @ Wout + bout, Wout (bf16) streamed from DRAM.
"""

import numpy as np

V, E, H, L, S, B = 10000, 256, 512, 2, 256, 64
NCORES = 8
BL = B // NCORES          # batch per core = 8
T = S * BL                # tokens per core = 2048
BLK = 16                  # recurrence pipeline block (steps)
NBLK = S // BLK
G3 = 3 * H                # 1536 (r|z|c gate width)
NG = G3 // 128            # 12 gate chunks of 128
VC = 500                  # logits N-chunk (10000 = 20*500, fits one PSUM bank)
NVC = V // VC
KO_H = H // 128           # 4
KO_E = E // 128           # 2

_CACHE = {}


def _build(phases="123", p2_reps=1, p2_loop=0):
    import concourse.mybir as mybir
    from concourse import bacc
    from concourse.tile import TileContext

    f32 = mybir.dt.float32
    bf16 = mybir.dt.bfloat16
    AF = mybir.ActivationFunctionType

    nc = bacc.Bacc("TRN2", target_bir_lowering=False, debug=False)

    # ---- DRAM I/O ----
    x0t_d = nc.dram_tensor("x0t", [E, T], bf16, kind="ExternalInput")
    h0t_init_d = nc.dram_tensor("h0t_init", [H, BL], f32, kind="ExternalInput")
    h1t_init_d = nc.dram_tensor("h1t_init", [H, BL], f32, kind="ExternalInput")
    w0x_d = nc.dram_tensor("w0x", [E, G3], bf16, kind="ExternalInput")
    w0hrz_d = nc.dram_tensor("w0hrz", [H, 2 * H], bf16, kind="ExternalInput")
    w0hc_d = nc.dram_tensor("w0hc", [H, H], bf16, kind="ExternalInput")
    w1x_d = nc.dram_tensor("w1x", [H, G3], bf16, kind="ExternalInput")
    w1hrz_d = nc.dram_tensor("w1hrz", [H, 2 * H], bf16, kind="ExternalInput")
    w1hc_d = nc.dram_tensor("w1hc", [H, H], bf16, kind="ExternalInput")
    b0t_d = nc.dram_tensor("b0t", [128, NG], f32, kind="ExternalInput")
    b1t_d = nc.dram_tensor("b1t", [128, NG], f32, kind="ExternalInput")
    wout_d = nc.dram_tensor("wout", [H, V], mybir.dt.float32r, kind="ExternalInput")
    bout_d = nc.dram_tensor("boutbc", [128, V], f32, kind="ExternalInput")

    logits_d = nc.dram_tensor("logits", [T, V], f32, kind="ExternalOutput")
    # hidden state output, transposed: [L, H, BL]
    hidden_d = nc.dram_tensor("hidden", [L, H, BL], f32, kind="ExternalOutput")

    # DRAM scratch for per-step transposed input-gate tiles [S, NG, 128, BL]
    gx0_d = nc.dram_tensor("gx0_scratch", [S, NG, 128, BL], f32)
    gx1_d = nc.dram_tensor("gx1_scratch", [S, NG, 128, BL], f32)

    with TileContext(nc) as tc:
        with (
            tc.tile_pool(name="persist", bufs=1) as pp,
            tc.tile_pool(name="ps", bufs=2, space="PSUM") as ps,
        ):
            # h1 transposed history (f32r) — lhsT of the logits matmul
            f32r = mybir.dt.float32r
            h1t_all = pp.tile([128, KO_H, T], f32r, tag="h1t_all")
            # layer-0 hT block buffers (bf16 ring) — rhs for gx1 blocks and
            # the per-step matmul input
            h0t_blocks = [pp.tile([128, KO_H, BLK * BL], bf16, tag=f"h0tb{i}",
                                  name=f"h0tb{i}") for i in range(2)]

            # ================= P1: gx0T = (X0 @ W0x + b0)^T =================
            with tc.tile_pool(name="p1", bufs=1) as p1p, \
                 tc.tile_pool(name="p1w", bufs=3) as p1w:
                if "1" in phases:
                    x0t = p1p.tile([128, KO_E, T], bf16, tag="x0t")
                    nc.sync.dma_start(
                        x0t[:], x0t_d.rearrange("(ko p) t -> p ko t", p=128))
                    w0x = p1p.tile([128, KO_E, G3], bf16, tag="w0x")
                    nc.sync.dma_start(
                        w0x[:], w0x_d.rearrange("(ko p) n -> p ko n", p=128))
                    b0t = p1p.tile([128, NG], f32, tag="b0t")
                    nc.sync.dma_start(b0t[:], b0t_d[:])
                    for nk in range(T // 512):    # token chunk (free) outer so
                        for mo in range(NG):      # early steps unblock first
                            gps = ps.tile([128, 512], f32, tag="mm1",
                                          name=f"p1ps{mo}_{nk}")
                            for ko in range(KO_E):
                                nc.tensor.matmul(
                                    gps[:], w0x[:, ko, mo * 128:(mo + 1) * 128],
                                    x0t[:, ko, nk * 512:(nk + 1) * 512],
                                    start=(ko == 0), stop=(ko == KO_E - 1))
                            gt = p1w.tile([128, 512], f32, tag="p1g",
                                          name=f"p1g{mo}_{nk}")
                            nc.scalar.activation(gt[:], gps[:], AF.Identity,
                                                 bias=b0t[:, mo:mo + 1])
                            # 512 tokens = 64 steps x 8 batch
                            t0 = nk * 64
                            nc.sync.dma_start(
                                gx0_d[t0:t0 + 64, mo].rearrange("t p b -> p t b"),
                                gt[:].rearrange("p (t b) -> p t b", b=BL))

            # ================= P2: pipelined recurrence =================
            with (
                tc.tile_pool(name="wrec", bufs=1) as wp,
                tc.tile_pool(name="state", bufs=3) as spool,
                tc.tile_pool(name="gxin", bufs=4) as gpool,
                tc.tile_pool(name="work", bufs=3) as wkpool,
                tc.tile_pool(name="gxblk", bufs=2) as gbpool,
            ):
                w0hrz = wp.tile([128, KO_H, 2 * H], bf16, tag="w0hrz")
                nc.sync.dma_start(
                    w0hrz[:], w0hrz_d.rearrange("(ko p) n -> p ko n", p=128))
                w0hc = wp.tile([128, KO_H, H], bf16, tag="w0hc")
                nc.sync.dma_start(
                    w0hc[:], w0hc_d.rearrange("(ko p) n -> p ko n", p=128))
                w1hrz = wp.tile([128, KO_H, 2 * H], bf16, tag="w1hrz")
                nc.sync.dma_start(
                    w1hrz[:], w1hrz_d.rearrange("(ko p) n -> p ko n", p=128))
                w1hc = wp.tile([128, KO_H, H], bf16, tag="w1hc")
                nc.sync.dma_start(
                    w1hc[:], w1hc_d.rearrange("(ko p) n -> p ko n", p=128))
                w1x = wp.tile([128, KO_H, G3], bf16, tag="w1x")
                nc.sync.dma_start(
                    w1x[:], w1x_d.rearrange("(ko p) n -> p ko n", p=128))
                b1t = wp.tile([128, NG], f32, tag="b1t")
                nc.sync.dma_start(b1t[:], b1t_d[:])

                # initial state: fp32 [128, KO_H, BL] + bf16 cast
                h0t_f = wp.tile([128, KO_H, BL], f32, tag="h0t_f0")
                nc.sync.dma_start(
                    h0t_f[:], h0t_init_d.rearrange("(ko p) b -> p ko b", p=128))
                h1t_f = wp.tile([128, KO_H, BL], f32, tag="h1t_f0")
                nc.sync.dma_start(
                    h1t_f[:], h1t_init_d.rearrange("(ko p) b -> p ko b", p=128))
                h0t_b0 = wp.tile([128, KO_H, BL], bf16, tag="h0t_b0")
                nc.vector.tensor_copy(h0t_b0[:], h0t_f[:])
                h1t_b0 = wp.tile([128, KO_H, BL], bf16, tag="h1t_b0")
                nc.vector.tensor_copy(h1t_b0[:], h1t_f[:])

                pfx = [""]
                gx_d = [gx0_d, gx1_d]
                whrz = [w0hrz, w1hrz]
                whc = [w0hc, w1hc]
                # state: (hT fp32 tile, hT bf16 AP)
                state = {0: (h0t_f, h0t_b0), 1: (h1t_f, h1t_b0)}

                def emit_step(layer, t):
                    hTf, hTb = state[layer]
                    gx = gpool.tile([128, NG, BL], f32, tag=f"gx{layer}",
                                    name=f"{pfx[0]}gx{layer}_{t}")
                    nc.sync.dma_start(
                        gx[:], gx_d[layer][t].rearrange("j p b -> p j b"))

                    # rzT [128, 8, BL] = Whrz^T-chunks @ hT  (+ gxT, sigmoid)
                    # r's 4 chunks first — they gate the cand matmul; z's
                    # matmuls and sigmoid run off the critical path.
                    r_ps = ps.tile([128, 4 * BL], f32, tag="rp",
                                   name=f"{pfx[0]}rps{layer}_{t}")
                    z_ps = ps.tile([128, 4 * BL], f32, tag="zp",
                                   name=f"{pfx[0]}zps{layer}_{t}")
                    rz = wkpool.tile([128, 8, BL], f32, tag="rz",
                                     name=f"{pfx[0]}rz{layer}_{t}")
                    for mo in range(4):
                        for ko in range(KO_H):
                            nc.tensor.matmul(
                                r_ps[:, mo * BL:(mo + 1) * BL],
                                whrz[layer][:, ko, mo * 128:(mo + 1) * 128],
                                hTb[:, ko],
                                start=(ko == 0), stop=(ko == KO_H - 1))
                    nc.vector.tensor_add(
                        rz[:, 0:4],
                        r_ps[:].rearrange("p (j b) -> p j b", b=BL), gx[:, 0:4])
                    nc.scalar.activation(rz[:, 0:4], rz[:, 0:4], AF.Sigmoid)

                    # rhT = rT * hT  (bf16, next matmul input)
                    rh = wkpool.tile([128, KO_H, BL], bf16, tag="rh",
                                     name=f"{pfx[0]}rh{layer}_{t}")
                    nc.vector.tensor_mul(rh[:], rz[:, 0:4], hTf[:])

                    for mo in range(4):
                        for ko in range(KO_H):
                            nc.tensor.matmul(
                                z_ps[:, mo * BL:(mo + 1) * BL],
                                whrz[layer][:, ko, (4 + mo) * 128:(5 + mo) * 128],
                                hTb[:, ko],
                                start=(ko == 0), stop=(ko == KO_H - 1))
                    nc.vector.tensor_add(
                        rz[:, 4:8],
                        z_ps[:].rearrange("p (j b) -> p j b", b=BL), gx[:, 4:8])
                    nc.scalar.activation(rz[:, 4:8], rz[:, 4:8], AF.Sigmoid)

                    # cT [128, 4, BL] = Whc^T-chunks @ rhT (+ gxT, tanh)
                    c_ps = ps.tile([128, 4 * BL], f32, tag="c",
                                   name=f"{pfx[0]}cps{layer}_{t}")
                    for mo in range(KO_H):
                        for ko in range(KO_H):
                            nc.tensor.matmul(
                                c_ps[:, mo * BL:(mo + 1) * BL],
                                whc[layer][:, ko, mo * 128:(mo + 1) * 128],
                                rh[:, ko],
                                start=(ko == 0), stop=(ko == KO_H - 1))
                    c = wkpool.tile([128, KO_H, BL], f32, tag="c",
                                    name=f"{pfx[0]}c{layer}_{t}")
                    nc.vector.tensor_add(
                        c[:], c_ps[:].rearrange("p (j b) -> p j b", b=BL),
                        gx[:, 8:12])
                    nc.scalar.activation(c[:], c[:], AF.Tanh)

                    # hT' = hT + zT*(cT - hT)
                    d = wkpool.tile([128, KO_H, BL], f32, tag="d",
                                    name=f"{pfx[0]}d{layer}_{t}")
                    nc.vector.tensor_sub(d[:], c[:], hTf[:])
                    nc.vector.tensor_mul(d[:], rz[:, 4:8], d[:])
                    if layer == 0:
                        hTf_new = spool.tile([128, KO_H, BL], f32, tag="h0",
                                             name=f"{pfx[0]}h0_{t}")
                    else:
                        # layer 1: blend writes the f32r logits history directly
                        hTf_new = h1t_all[:, :, t * BL:(t + 1) * BL]
                    nc.vector.tensor_add(hTf_new[:], hTf[:], d[:])

                    # bf16 cast (= next step's matmul rhs)
                    if layer == 0:
                        blk, off = t // BLK, (t % BLK) * BL
                        dst = h0t_blocks[blk % 2][:, :, off:off + BL]
                    else:
                        dst = spool.tile([128, KO_H, BL], bf16, tag="h1b",
                                         name=f"{pfx[0]}h1b_{t}")
                    nc.vector.tensor_copy(dst, hTf_new[:])

                    state[layer] = (hTf_new, dst)
                    if t == S - 1:
                        src_ap = hTf_new[:]
                        if src_ap.dtype != f32:
                            src_ap = src_ap.bitcast(f32)
                        nc.sync.dma_start(
                            hidden_d[layer].rearrange("(ko p) b -> p ko b",
                                                      p=128), src_ap)

                def emit_gx1_block(k):
                    src = h0t_blocks[k % 2]
                    gt = gbpool.tile([128, NG, BLK * BL], f32, tag="g1big",
                                     name=f"{pfx[0]}g1b{k}")
                    for mo in range(NG):
                        gps = ps.tile([128, BLK * BL], f32, tag="c",
                                      name=f"{pfx[0]}g1ps{k}_{mo}")
                        for ko in range(KO_H):
                            nc.tensor.matmul(
                                gps[:], w1x[:, ko, mo * 128:(mo + 1) * 128],
                                src[:, ko],
                                start=(ko == 0), stop=(ko == KO_H - 1))
                        nc.scalar.activation(gt[:, mo], gps[:], AF.Identity,
                                             bias=b1t[:, mo:mo + 1])
                    t0 = k * BLK
                    for mo in range(NG):
                        nc.sync.dma_start(
                            gx1_d[t0:t0 + BLK, mo].rearrange("t p b -> p t b"),
                            gt[:, mo].rearrange("p (t b) -> p t b", b=BL))

                if "L" in phases:       # debug: single-layer stream
                    for t in range(S):
                        emit_step(0, t)
                if "2" in phases and p2_loop:
                    import contextlib
                    with tc.For_i(0, p2_loop, 1):
                        state[0] = (h0t_f, h0t_b0)
                        state[1] = (h1t_f, h1t_b0)
                        for t in range(BLK):
                            emit_step(0, t)
                        emit_gx1_block(0)
                        for k in range(1, NBLK):
                            for i in range(BLK):
                                emit_step(0, k * BLK + i)
                                emit_step(1, (k - 1) * BLK + i)
                            emit_gx1_block(k)
                        for i in range(BLK):
                            emit_step(1, (NBLK - 1) * BLK + i)
                elif "2" in phases:
                    for rep in range(p2_reps):
                        state[0] = (h0t_f, h0t_b0)
                        state[1] = (h1t_f, h1t_b0)
                        pfx[0] = f"r{rep}_" if rep else ""
                        for t in range(BLK):
                            emit_step(0, t)
                        emit_gx1_block(0)
                        for k in range(1, NBLK):
                            for i in range(BLK):
                                emit_step(0, k * BLK + i)
                                emit_step(1, (k - 1) * BLK + i)
                            emit_gx1_block(k)
                        for i in range(BLK):
                            emit_step(1, (NBLK - 1) * BLK + i)

            # ================= P3: logits =================
            with (
                tc.tile_pool(name="woutp", bufs=3) as wopool,
                tc.tile_pool(name="outp", bufs=4) as opool,
                tc.tile_pool(name="boutp", bufs=2) as bopool,
            ):
                for nck in (range(NVC) if "3" in phases else []):
                    wo = wopool.tile([128, KO_H, VC], f32r, tag="wo",
                                     name=f"wo{nck}")
                    nc.sync.dma_start(
                        wo[:],
                        wout_d[:, nck * VC:(nck + 1) * VC]
                        .rearrange("(ko p) n -> p ko n", p=128))
                    bo = bopool.tile([128, VC], f32, tag="bo", name=f"bo{nck}")
                    nc.sync.dma_start(bo[:], bout_d[:, nck * VC:(nck + 1) * VC])
                    for m in range(T // 128):
                        lps = ps.tile([128, VC], f32, tag="mm1",
                                      name=f"lps{nck}_{m}")
                        for ko in range(KO_H):
                            nc.tensor.matmul(
                                lps[:], h1t_all[:, ko, m * 128:(m + 1) * 128],
                                wo[:, ko],
                                start=(ko == 0), stop=(ko == KO_H - 1))
                        ot = opool.tile([128, VC], f32, tag="ot",
                                        name=f"ot{nck}_{m}")
                        nc.vector.tensor_add(ot[:], lps[:], bo[:])
                        nc.sync.dma_start(
                            logits_d[m * 128:(m + 1) * 128,
                                     nck * VC:(nck + 1) * VC], ot[:])

    nc.compile()
    return nc


def _prep_host(inputs):
    """Host-side prep: embedding gather, weight concat/cast, per-core shard."""
    import ml_dtypes
    bf = ml_dtypes.bfloat16

    inp = {k: np.asarray(v) for k, v in inputs.items()}
    tok = inp["inputs"].astype(np.int64)          # [S, B]
    embW = inp["emb_W"].astype(np.float32)
    X0 = embW[tok]                                # [S, B, E]
    hidden = inp["hidden"].astype(np.float32)     # [L, B, H]

    def cat(*ws):
        return np.ascontiguousarray(
            np.concatenate(ws, axis=1).astype(np.float32)).astype(bf)

    w0x = cat(inp["Wr0"][:E], inp["Wz0"][:E], inp["Wh0"][:E])
    w0hrz = cat(inp["Wr0"][E:], inp["Wz0"][E:])
    w0hc = np.ascontiguousarray(inp["Wh0"][E:].astype(np.float32)).astype(bf)
    w1x = cat(inp["Wr1"][:H], inp["Wz1"][:H], inp["Wh1"][:H])
    w1hrz = cat(inp["Wr1"][H:], inp["Wz1"][H:])
    w1hc = np.ascontiguousarray(inp["Wh1"][H:].astype(np.float32)).astype(bf)
    b0 = np.concatenate([inp["br0"], inp["bz0"], inp["bh0"]]).astype(np.float32)
    b1 = np.concatenate([inp["br1"], inp["bz1"], inp["bh1"]]).astype(np.float32)
    b0t = np.ascontiguousarray(b0.reshape(NG, 128).T)     # [128, NG]
    b1t = np.ascontiguousarray(b1.reshape(NG, 128).T)
    wout = np.ascontiguousarray(inp["Wout"].astype(np.float32))
    boutbc = np.ascontiguousarray(
        np.broadcast_to(inp["bout"].astype(np.float32), (128, V)))

    shared = dict(w0x=w0x, w0hrz=w0hrz, w0hc=w0hc, w1x=w1x, w1hrz=w1hrz,
                  w1hc=w1hc, b0t=b0t, b1t=b1t, wout=wout, boutbc=boutbc)

    in_maps = []
    for c in range(NCORES):
        bs = slice(c * BL, (c + 1) * BL)
        x0c = X0[:, bs, :].reshape(T, E)          # [T, E]
        x0t = np.ascontiguousarray(x0c.T).astype(bf)   # [E, T] bf16
        h0n = np.ascontiguousarray(hidden[0, bs, :])   # [BL, H]
        h1n = np.ascontiguousarray(hidden[1, bs, :])
        m = dict(shared)
        m.update(
            x0t=x0t,
            h0t_init=np.ascontiguousarray(h0n.T),
            h1t_init=np.ascontiguousarray(h1n.T),
        )
        in_maps.append(m)
    return in_maps


def kernel(**inputs):
    from concourse.bass_utils import run_bass_kernel_spmd

    if "nc" not in _CACHE:
        _CACHE["nc"] = _build()
    nc = _CACHE["nc"]

    in_maps = _prep_host(inputs)
    res = run_bass_kernel_spmd(nc, in_maps, core_ids=list(range(NCORES)))

    logits = np.empty((S, B, V), dtype=np.float32)
    hidden_final = np.empty((L, B, H), dtype=np.float32)
    for c in range(NCORES):
        bs = slice(c * BL, (c + 1) * BL)
        logits[:, bs, :] = res.results[c]["logits"].reshape(S, BL, V)
        # hidden comes back transposed [L, H, BL]
        hidden_final[:, bs, :] = res.results[c]["hidden"].transpose(0, 2, 1)
    return logits, hidden_final
